# revision 9
# baseline (speedup 1.0000x reference)
"""BiDAF attention kernel for 8 Trainium2 NeuronCores.

Data-parallel over batch (B=32 -> 4 per core). Batches processed in pairs so
the Lq=64-row elementwise/transpose work fills all 128 partitions. Per batch:
  sT[j,i] = (q*cqw) @ c^T + s0[i] + s1[j] + bias   (bf16 matmuls, fp32 accum;
  s0 comes free as rows 64/96 of the same matmul via cwgt hi/lo columns, then
  a rank-1 fp32 matmul broadcasts it across rows)
  E = exp(sT)  (one exp serves both softmaxes; s1+bias fused via act bias)
  a2T = E / rowsum(E);  a1 normalization deferred: 1/colsum(E) applied to
  output rows of downstream matmuls.
  a|b = E_m^T @ [q | M2] in one 1024-wide matmul; M2 = a2^T @ c.
  out = [c, a, c*a, c*b]
DMA: input loads dispatched first on the sync queue, c-passthrough stores on
the scalar queue, m-tile stores stream on sync behind compute.
"""

import sys

if "/opt/trn_rl_repo" not in sys.path:
    sys.path.insert(0, "/opt/trn_rl_repo")

from contextlib import ExitStack

import numpy as np

import concourse.bacc as bacc
import concourse.bass as bass
import concourse.mybir as mybir
from concourse.bass import ts
from concourse.bass_utils import run_bass_kernel_spmd
from concourse.masks import make_identity
from concourse.tile import TileContext

N_CORES = 8
B, Lc, Lq, H = 32, 512, 64, 512
BPC = B // N_CORES  # batches per core
F32 = mybir.dt.float32
BF16 = mybir.dt.bfloat16
MULT = mybir.AluOpType.mult

_CACHE = {}


def _build_program():
    nc = bacc.Bacc("TRN2", target_bir_lowering=False, debug=False, num_devices=N_CORES)
    c_h = nc.dram_tensor("c", [BPC, Lc, H], F32, kind="ExternalInput")
    q_h = nc.dram_tensor("q", [BPC, Lq, H], F32, kind="ExternalInput")
    cqw_h = nc.dram_tensor("cqw", [H], F32, kind="ExternalInput")
    cwgt_h = nc.dram_tensor("cwgt", [H], F32, kind="ExternalInput")
    qwgt_h = nc.dram_tensor("qwgt", [H], F32, kind="ExternalInput")
    bias_h = nc.dram_tensor("bias", [1], F32, kind="ExternalInput")
    out_h = nc.dram_tensor("out", [BPC, Lc, 4 * H], F32, kind="ExternalOutput")

    c_ap = c_h.ap()
    q_ap = q_h.ap()
    out_ap = out_h.ap()

    exp_f = mybir.ActivationFunctionType.Exp
    ident_f = mybir.ActivationFunctionType.Identity
    copy_f = mybir.ActivationFunctionType.Copy

    with TileContext(nc) as tc, ExitStack() as ctx:
        const = ctx.enter_context(tc.tile_pool(name="const", bufs=1))
        cpool = ctx.enter_context(tc.tile_pool(name="cpool", bufs=4))
        cbpool = ctx.enter_context(tc.tile_pool(name="cbpool", bufs=3))
        ctpool = ctx.enter_context(tc.tile_pool(name="ctpool", bufs=3))
        qpool = ctx.enter_context(tc.tile_pool(name="qpool", bufs=2))
        spool = ctx.enter_context(tc.tile_pool(name="spool", bufs=2))
        lpool = ctx.enter_context(tc.tile_pool(name="lpool", bufs=3))
        epool = ctx.enter_context(tc.tile_pool(name="epool", bufs=2))
        btpool = ctx.enter_context(tc.tile_pool(name="btpool", bufs=2))
        opool = ctx.enter_context(tc.tile_pool(name="opool", bufs=4))
        ps_tr = ctx.enter_context(tc.tile_pool(name="ps_tr", bufs=1, space="PSUM"))
        ps_mm = ctx.enter_context(tc.tile_pool(name="ps_mm", bufs=2, space="PSUM"))
        ps_ab = ctx.enter_context(tc.tile_pool(name="ps_ab", bufs=2, space="PSUM"))
        ps_sm = ctx.enter_context(tc.tile_pool(name="ps_sm", bufs=1, space="PSUM"))

        NP = BPC // 2  # pairs per core

        # ---- input loads: dispatched first on the sync queue ----
        c_tiles = []
        q_pairs = []
        for p in range(NP):
            b0, b1 = 2 * p, 2 * p + 1
            c0 = cpool.tile([128, 4, H], F32, name="c_sb")
            nc.sync.dma_start(out=c0, in_=c_ap[b0].rearrange("(j p) h -> p j h", p=128))
            q_t = qpool.tile([128, H], F32, name="q_sb")
            nc.sync.dma_start(out=q_t[0:Lq, :], in_=q_ap[b0])
            nc.sync.dma_start(out=q_t[Lq:128, :], in_=q_ap[b1])
            c1 = cpool.tile([128, 4, H], F32, name="c_sb")
            nc.sync.dma_start(out=c1, in_=c_ap[b1].rearrange("(j p) h -> p j h", p=128))
            c_tiles += [c0, c1]
            q_pairs.append(q_t)

        # ---- constants (gpsimd queue; overlap the input loads) ----
        ident = const.tile([128, 128], BF16, name="ident")
        make_identity(nc, ident)
        cw_bc = const.tile([128, H], F32, name="cw_bc")  # cq_weight bcast over rows
        nc.gpsimd.dma_start(out=cw_bc, in_=bass.AP(tensor=cqw_h, offset=0, ap=[[0, 128], [1, H]]))
        qw_bc = const.tile([128, H], F32, name="qw_bc")  # q_weight bcast over rows
        nc.gpsimd.dma_start(out=qw_bc, in_=bass.AP(tensor=qwgt_h, offset=0, ap=[[0, 128], [1, H]]))
        cwgt_col = const.tile([128, 4], F32, name="cwgt_col")  # c_weight as 4 chunks
        nc.gpsimd.dma_start(out=cwgt_col, in_=bass.AP(tensor=cwgt_h, offset=0, ap=[[1, 128], [128, 4]]))
        cwgt_hi = const.tile([128, 4], BF16, name="cwgt_hi")
        nc.vector.tensor_copy(out=cwgt_hi, in_=cwgt_col)
        cwgt_res = const.tile([128, 4], F32, name="cwgt_res")
        nc.vector.tensor_sub(cwgt_res, cwgt_col, cwgt_hi)
        cwgt_lo = const.tile([128, 4], BF16, name="cwgt_lo")
        nc.vector.tensor_copy(out=cwgt_lo, in_=cwgt_res)
        bias_bc = const.tile([128, 1], F32, name="bias_bc")
        nc.gpsimd.dma_start(out=bias_bc, in_=bass.AP(tensor=bias_h, offset=0, ap=[[0, 128], [1, 1]]))
        ones_col = const.tile([128, 1], BF16, name="ones_col")
        nc.vector.memset(ones_col, 1.0)
        aug_f = const.tile([1, 97], F32, name="aug_f")
        nc.vector.memset(aug_f[:, 0:64], 1.0)
        nc.vector.memset(aug_f[:, 64:97], 0.0)
        aug = const.tile([1, 97], mybir.dt.float32r, name="aug")  # rank-1 s0 add
        nc.vector.tensor_copy(out=aug, in_=aug_f)

        P = [dict() for _ in range(NP)]  # per-pair tile state
        S = [dict() for _ in range(BPC)]  # per-batch tile state

        def stage_PQ(p):
            """pair-level q-side prep: casts, qs, s1, qT transposes, lhsT."""
            q_sb = q_pairs[p]
            # qm2 pair tile: cols 0:512 = q (bf16), cols 512:1024 = M2 (later)
            qm2 = qpool.tile([128, 2 * H], BF16, name="qm2")
            nc.gpsimd.tensor_copy(out=qm2[:, 0:H], in_=q_sb)
            # qs = q * cq_weight (bf16 out) ; s1 = (q @ q_weight) + bias
            qs_bf = qpool.tile([128, H], BF16, name="qs_bf")
            nc.vector.tensor_mul(qs_bf, q_sb, cw_bc)
            s1_scr = qpool.tile([128, H], F32, name="s1_scr")
            s1_raw = spool.tile([128, 1], F32, name="s1_raw")
            nc.gpsimd.tensor_mul(s1_scr, q_sb, qw_bc)
            nc.vector.tensor_reduce(
                out=s1_raw, in_=s1_scr, axis=mybir.AxisListType.X,
                op=mybir.AluOpType.add,
            )
            s1b = spool.tile([128, 1], F32, name="s1b")
            nc.scalar.activation(out=s1b, in_=s1_raw, func=ident_f, bias=bias_bc, scale=1.0)

            # qT chunks for the pair: pt_q[:, f, 0:64] = b0, [:, f, 64:128] = b1
            pt_q = ps_tr.tile([128, 4, 128], BF16, name="pt_q", tag="trq", bufs=1)
            for f in range(4):
                nc.tensor.transpose(pt_q[:, f, :], qs_bf[:, ts(f, 128)], ident)
            # lhsT[b][f] = [ (qs_b chunk f)^T | cwgt_hi f | junk | cwgt_lo f ]
            # (hi lands in psum row 64, lo in row 96: engine reads need
            # 32-aligned base partitions; rows 65..95 are unused junk)
            for b in (2 * p, 2 * p + 1):
                off = (b % 2) * 64
                lhsT = lpool.tile([128, 4, 97], BF16, name="lhsT")
                nc.vector.tensor_copy(out=lhsT[:, :, 0:64], in_=pt_q[:, :, off:off + 64])
                nc.vector.tensor_copy(out=lhsT[:, :, 64:65], in_=cwgt_hi.rearrange("p (f o) -> p f o", o=1))
                nc.vector.tensor_copy(out=lhsT[:, :, 96:97], in_=cwgt_lo.rearrange("p (f o) -> p f o", o=1))
                S[b]["lhsT"] = lhsT
            P[p].update(qm2=qm2, s1b=s1b)

        def stage_A(b):
            """per-batch: c cast -> cT transposes -> sT matmuls -> exp"""
            p, off = b // 2, (b % 2) * 64
            c_sb = c_tiles[b]
            c_bf = cbpool.tile([128, 4, H], BF16, name="c_bf")
            nc.scalar.activation(out=c_bf[:, 0, :], in_=c_sb[:, 0, :], func=copy_f)
            nc.gpsimd.tensor_copy(out=c_bf[:, 1, :], in_=c_sb[:, 1, :])
            nc.scalar.activation(out=c_bf[:, 2, :], in_=c_sb[:, 2, :], func=copy_f)
            nc.gpsimd.tensor_copy(out=c_bf[:, 3, :], in_=c_sb[:, 3, :])
            # c passthrough store on the scalar queue (depends only on the load)
            nc.scalar.dma_start(
                out=out_ap[b, :, 0:512].rearrange("(j p) h -> p j h", p=128),
                in_=c_sb,
            )

            # cT[f] = c^T chunk (H rows f*128.., all Lc cols), bf16
            cT = ctpool.tile([128, 4, H], BF16, name="cT")
            for j in range(4):
                pt_c = ps_mm.tile([128, 4, 128], BF16, name="pt_c", tag="big1")
                for f in range(4):
                    nc.tensor.transpose(pt_c[:, f, :], c_bf[:, j, ts(f, 128)], ident)
                if j % 2 == 0:
                    nc.vector.tensor_copy(out=cT[:, :, ts(j, 128)], in_=pt_c)
                else:
                    nc.scalar.activation(out=cT[:, :, ts(j, 128)], in_=pt_c, func=copy_f)

            # sT accumulation: rows 0..63 = qs@cT, rows 64/96 = s0 hi/lo parts
            lhsT = S[b].pop("lhsT")
            ps_sT = ps_mm.tile([128, 512], F32, name="ps_sT", tag="big1", bufs=2)
            for f in range(4):
                nc.tensor.matmul(
                    ps_sT[0:97, :], lhsT[:, f, :], cT[:, f, :],
                    start=(f == 0), stop=False,
                )
            s0hi = spool.tile([1, H], F32, name="s0hi")
            nc.scalar.activation(out=s0hi, in_=ps_sT[64:65, :], func=copy_f)
            s0row = spool.tile([1, H], mybir.dt.float32r, name="s0row")
            nc.vector.tensor_add(s0row, ps_sT[96:97, :], s0hi)
            nc.tensor.matmul(
                ps_sT[0:97, :], aug, s0row,
                start=False, stop=True,
            )

            # E = exp(sT + s1 + bias) in bf16; rowsum (f32) for a2
            if off == 0:
                E_pair = epool.tile([128, H], BF16, name="E_pair")
                rowsum = spool.tile([128, 1], F32, name="rowsum")
                P[p].update(E_pair=E_pair, rowsum=rowsum)
            else:
                E_pair = P[p]["E_pair"]
                rowsum = P[p]["rowsum"]
            nc.scalar.activation(
                out=E_pair[off:off + 64, :], in_=ps_sT[0:64, :], func=exp_f,
                bias=P[p]["s1b"][off:off + 64, :], scale=1.0,
                accum_out=rowsum[off:off + 64, :],
            )
            S[b].update(c_sb=c_sb, c_bf=c_bf)

        def stage_B(p):
            """pair: a2 softmax -> a2 transposes -> M2 = a2^T @ c ; colsums"""
            E_pair = P[p]["E_pair"]
            ra2 = spool.tile([128, 1], F32, name="ra2")
            nc.vector.reciprocal(ra2, P[p]["rowsum"])
            a2T = epool.tile([128, H], BF16, name="a2T")
            nc.vector.tensor_scalar_mul(a2T, E_pair, ra2)

            # a2 natural layout [i, j-pair] via PE transposes of a2T
            a2n = btpool.tile([128, 4, 128], BF16, name="a2n")
            pt_a = ps_tr.tile([128, 4, 128], BF16, name="pt_a", tag="trq", bufs=1)
            for f in range(4):
                nc.tensor.transpose(pt_a[:, f, :], a2T[:, ts(f, 128)], ident)
            nc.vector.tensor_copy(out=a2n, in_=pt_a)

            # M2 = a2^T @ c for both batches into one psum pair tile
            ps_M2 = ps_mm.tile([128, 512], F32, name="ps_M2", tag="big1", bufs=2)
            for b in (2 * p, 2 * p + 1):
                off = (b % 2) * 64
                c_bf = S[b]["c_bf"]
                for jj in range(4):
                    nc.tensor.matmul(
                        ps_M2[off:off + 64, :], a2n[:, jj, off:off + 64], c_bf[:, jj, :],
                        start=(jj == 0), stop=(jj == 3),
                    )
            nc.vector.tensor_copy(out=P[p]["qm2"][:, H:2 * H], in_=ps_M2)

            # column sums of E (normalizer of a1), reciprocal per i-tile
            for b in (2 * p, 2 * p + 1):
                off = (b % 2) * 64
                ps_S = ps_sm.tile([128, 4], F32, name="ps_S")
                for m in range(4):
                    nc.tensor.matmul(
                        ps_S[:, m:m + 1], E_pair[off:off + 64, ts(m, 128)],
                        ones_col[off:off + 64, :], start=True, stop=True,
                    )
                rS = spool.tile([128, 4], F32, name="rS")
                nc.vector.reciprocal(rS, ps_S)
                S[b]["rS"] = rS

        def stage_C(b, ms):
            """per i-tile: [a|b] matmul, scales, products, store"""
            p, off = b // 2, (b % 2) * 64
            c_sb = S[b]["c_sb"]
            E_pair = P[p]["E_pair"]
            qm2 = P[p]["qm2"]
            rS = S[b]["rS"]
            for m in ms:
                stage = opool.tile([128, 3, H], F32, name="stage")
                ps = ps_ab.tile([128, 2 * H], F32, name="ps", tag="big2")
                nc.tensor.matmul(
                    ps[:, 0:H], E_pair[off:off + 64, ts(m, 128)], qm2[off:off + 64, 0:H],
                    start=True, stop=True,
                )
                nc.tensor.matmul(
                    ps[:, H:2 * H], E_pair[off:off + 64, ts(m, 128)], qm2[off:off + 64, H:2 * H],
                    start=True, stop=True,
                )
                # a = (E^T chunk @ q) * rS ; ca = c * a
                nc.scalar.activation(out=stage[:, 0, :], in_=ps[:, 0:H], func=copy_f, scale=rS[:, m:m + 1])
                if m % 2 == 0:
                    nc.vector.tensor_mul(stage[:, 1, :], stage[:, 0, :], c_sb[:, m, :])
                else:
                    nc.gpsimd.tensor_mul(stage[:, 1, :], stage[:, 0, :], c_sb[:, m, :])
                # b = (a1 @ M2) * rS ; cb = c * b
                nc.vector.scalar_tensor_tensor(
                    out=stage[:, 2, :], in0=ps[:, H:2 * H], scalar=rS[:, m:m + 1],
                    in1=c_sb[:, m, :], op0=MULT, op1=MULT,
                )
                # store: out tile = [a | c*a | c*b]
                nc.sync.dma_start(out=out_ap[b, ts(m, 128), 512:2048], in_=stage)
            if ms[-1] == 3:
                S[b].clear()

        # software-pipelined emission over the two pairs
        stage_PQ(0)
        stage_A(0)
        stage_A(1)
        stage_B(0)
        stage_C(0, [0, 1])
        stage_PQ(1)
        stage_C(0, [2, 3])
        stage_A(2)
        stage_C(1, [0, 1])
        stage_A(3)
        stage_C(1, [2, 3])
        stage_B(1)
        stage_C(2, [0, 1, 2, 3])
        stage_C(3, [0, 1, 2, 3])

    nc.compile()
    return nc


def _numpy_fallback(c, q, c_mask, q_mask, c_weight, q_weight, cq_weight, bias):
    NEG_INF = -1e30
    s0 = c @ c_weight
    s1 = (q @ q_weight).transpose(0, 2, 1)
    s2 = np.einsum("bih,bjh->bij", c * cq_weight, q)
    s = s0 + s1 + s2 + bias

    def softmax(x, mask, axis):
        logits = np.where(mask, x, NEG_INF)
        m = logits.max(axis=axis, keepdims=True)
        e = np.exp(logits - m)
        return e / e.sum(axis=axis, keepdims=True)

    a1 = softmax(s, q_mask[:, None, :], 2)
    a2 = softmax(s, c_mask[:, :, None], 1)
    a = np.einsum("bij,bjh->bih", a1, q)
    bb = np.einsum("bik,bjk->bij", a1, a2)
    bb = np.einsum("bij,bjh->bih", bb, c)
    return np.concatenate([c, a, c * a, c * bb], axis=2).astype(np.float32)


def kernel(c, q, c_mask, q_mask, c_weight, q_weight, cq_weight, bias, **_):
    c = np.asarray(c, dtype=np.float32)
    q = np.asarray(q, dtype=np.float32)
    if not (np.all(c_mask) and np.all(q_mask)):
        # masks are all-ones per the problem spec; keep a correct fallback
        return _numpy_fallback(
            c, q, np.asarray(c_mask), np.asarray(q_mask),
            np.asarray(c_weight, np.float32), np.asarray(q_weight, np.float32),
            np.asarray(cq_weight, np.float32), np.asarray(bias, np.float32),
        )

    if "nc" not in _CACHE:
        _CACHE["nc"] = _build_program()
    nc = _CACHE["nc"]

    cqw = np.ascontiguousarray(np.asarray(cq_weight, np.float32).reshape(H))
    cwgt = np.ascontiguousarray(np.asarray(c_weight, np.float32).reshape(H))
    qwgt = np.ascontiguousarray(np.asarray(q_weight, np.float32).reshape(H))
    bias_a = np.ascontiguousarray(np.asarray(bias, np.float32).reshape(1))

    in_maps = []
    for k in range(N_CORES):
        in_maps.append(
            {
                "c": np.ascontiguousarray(c[k * BPC : (k + 1) * BPC]),
                "q": np.ascontiguousarray(q[k * BPC : (k + 1) * BPC]),
                "cqw": cqw,
                "cwgt": cwgt,
                "qwgt": qwgt,
                "bias": bias_a,
            }
        )
    res = run_bass_kernel_spmd(nc, in_maps, core_ids=list(range(N_CORES)))
    return np.concatenate([res.results[k]["out"] for k in range(N_CORES)], axis=0)


# revision 14
# speedup vs baseline: 1.0290x; 1.0290x over previous
"""BiDAF attention kernel for 8 Trainium2 NeuronCores.

Data-parallel over batch (B=32 -> 4 per core). Per batch, on-chip:
  sT[j,i] = (q*cqw) @ c^T + s0[i] + s1[j] + bias   (bf16 matmuls, fp32 accum;
  s0 comes free as rows 64/96 of the same matmul via cwgt hi/lo columns, then
  a rank-1 fp32 matmul broadcasts it across rows)
  E = exp(sT)  (one exp serves both softmaxes; s1+bias fused via act bias)
  a2T = E / rowsum(E);  a1 normalization deferred: 1/colsum(E) applied to
  output rows of downstream matmuls.
  a = a1 @ q; b = a1 @ (a2^T @ c); out = [c, a, c*a, c*b]
q-side prep (casts, qs, s1, q^T) is batch-pair-packed to fill 128 partitions.
DMA: input loads dispatched first on the sync queue, c-passthrough stores on
the scalar queue, m-tile stores stream on sync behind compute.
"""

import sys

if "/opt/trn_rl_repo" not in sys.path:
    sys.path.insert(0, "/opt/trn_rl_repo")

from contextlib import ExitStack

import numpy as np

import concourse.bacc as bacc
import concourse.bass as bass
import concourse.mybir as mybir
from concourse.bass import ts
from concourse.bass_utils import run_bass_kernel_spmd
from concourse.masks import make_identity
from concourse.tile import TileContext

N_CORES = 8
B, Lc, Lq, H = 32, 512, 64, 512
BPC = B // N_CORES  # batches per core
F32 = mybir.dt.float32
BF16 = mybir.dt.bfloat16
MULT = mybir.AluOpType.mult

_CACHE = {}


def _build_program():
    nc = bacc.Bacc("TRN2", target_bir_lowering=False, debug=False, num_devices=N_CORES)
    c_h = nc.dram_tensor("c", [BPC, Lc, H], F32, kind="ExternalInput")
    q_h = nc.dram_tensor("q", [BPC, Lq, H], F32, kind="ExternalInput")
    cqw_h = nc.dram_tensor("cqw", [H], F32, kind="ExternalInput")
    cwgt_h = nc.dram_tensor("cwgt", [H], F32, kind="ExternalInput")
    qwgt_h = nc.dram_tensor("qwgt", [H], F32, kind="ExternalInput")
    bias_h = nc.dram_tensor("bias", [1], F32, kind="ExternalInput")
    out_h = nc.dram_tensor("out", [BPC, Lc, 4 * H], F32, kind="ExternalOutput")

    c_ap = c_h.ap()
    q_ap = q_h.ap()
    out_ap = out_h.ap()

    exp_f = mybir.ActivationFunctionType.Exp
    ident_f = mybir.ActivationFunctionType.Identity
    copy_f = mybir.ActivationFunctionType.Copy

    with TileContext(nc) as tc, ExitStack() as ctx:
        const = ctx.enter_context(tc.tile_pool(name="const", bufs=1))
        cpool = ctx.enter_context(tc.tile_pool(name="cpool", bufs=4))
        cbpool = ctx.enter_context(tc.tile_pool(name="cbpool", bufs=3))
        ctpool = ctx.enter_context(tc.tile_pool(name="ctpool", bufs=3))
        qpool = ctx.enter_context(tc.tile_pool(name="qpool", bufs=2))
        spool = ctx.enter_context(tc.tile_pool(name="spool", bufs=3))
        lpool = ctx.enter_context(tc.tile_pool(name="lpool", bufs=3))
        epool = ctx.enter_context(tc.tile_pool(name="epool", bufs=3))
        btpool = ctx.enter_context(tc.tile_pool(name="btpool", bufs=2))
        opool = ctx.enter_context(tc.tile_pool(name="opool", bufs=4))
        ps_tr = ctx.enter_context(tc.tile_pool(name="ps_tr", bufs=1, space="PSUM"))
        ps_mm = ctx.enter_context(tc.tile_pool(name="ps_mm", bufs=2, space="PSUM"))
        ps_ab = ctx.enter_context(tc.tile_pool(name="ps_ab", bufs=2, space="PSUM"))
        ps_sm = ctx.enter_context(tc.tile_pool(name="ps_sm", bufs=1, space="PSUM"))

        NP = BPC // 2  # pairs per core

        # ---- input loads: dispatched first on the sync queue ----
        c_tiles = []
        q_pairs = []
        for p in range(NP):
            b0, b1 = 2 * p, 2 * p + 1
            c0 = cpool.tile([128, 4, H], F32, name="c_sb")
            nc.sync.dma_start(out=c0, in_=c_ap[b0].rearrange("(j p) h -> p j h", p=128))
            q_t = qpool.tile([128, H], F32, name="q_sb")
            nc.sync.dma_start(out=q_t[0:Lq, :], in_=q_ap[b0])
            nc.sync.dma_start(out=q_t[Lq:128, :], in_=q_ap[b1])
            c1 = cpool.tile([128, 4, H], F32, name="c_sb")
            nc.sync.dma_start(out=c1, in_=c_ap[b1].rearrange("(j p) h -> p j h", p=128))
            c_tiles += [c0, c1]
            q_pairs.append(q_t)

        # ---- constants (gpsimd queue; overlap the input loads) ----
        ident = const.tile([128, 128], BF16, name="ident")
        make_identity(nc, ident)
        cw_bc = const.tile([128, H], F32, name="cw_bc")  # cq_weight bcast over rows
        nc.gpsimd.dma_start(out=cw_bc, in_=bass.AP(tensor=cqw_h, offset=0, ap=[[0, 128], [1, H]]))
        qw_bc = const.tile([128, H], F32, name="qw_bc")  # q_weight bcast over rows
        nc.gpsimd.dma_start(out=qw_bc, in_=bass.AP(tensor=qwgt_h, offset=0, ap=[[0, 128], [1, H]]))
        cwgt_col = const.tile([128, 4], F32, name="cwgt_col")  # c_weight as 4 chunks
        nc.gpsimd.dma_start(out=cwgt_col, in_=bass.AP(tensor=cwgt_h, offset=0, ap=[[1, 128], [128, 4]]))
        cwgt_hi = const.tile([128, 4], BF16, name="cwgt_hi")
        nc.vector.tensor_copy(out=cwgt_hi, in_=cwgt_col)
        cwgt_res = const.tile([128, 4], F32, name="cwgt_res")
        nc.vector.tensor_sub(cwgt_res, cwgt_col, cwgt_hi)
        cwgt_lo = const.tile([128, 4], BF16, name="cwgt_lo")
        nc.vector.tensor_copy(out=cwgt_lo, in_=cwgt_res)
        bias_bc = const.tile([128, 1], F32, name="bias_bc")
        nc.gpsimd.dma_start(out=bias_bc, in_=bass.AP(tensor=bias_h, offset=0, ap=[[0, 128], [1, 1]]))
        ones_col = const.tile([128, 1], BF16, name="ones_col")
        nc.vector.memset(ones_col, 1.0)
        aug_f = const.tile([1, 97], F32, name="aug_f")
        nc.vector.memset(aug_f[:, 0:64], 1.0)
        nc.vector.memset(aug_f[:, 64:97], 0.0)
        aug = const.tile([1, 97], mybir.dt.float32r, name="aug")  # rank-1 s0 add
        nc.vector.tensor_copy(out=aug, in_=aug_f)

        P = [dict() for _ in range(NP)]  # per-pair tile state
        S = [dict() for _ in range(BPC)]  # per-batch tile state

        def stage_PQ(p):
            """pair-level q-side prep: casts, qs, s1, qT transposes, lhsT."""
            q_sb = q_pairs[p]
            q_bf = qpool.tile([128, H], BF16, name="q_bf")
            nc.vector.tensor_copy(out=q_bf, in_=q_sb)
            # qs = q * cq_weight (bf16 out) ; s1 = (q @ q_weight) + bias
            qs_bf = qpool.tile([128, H], BF16, name="qs_bf")
            nc.vector.tensor_mul(qs_bf, q_sb, cw_bc)
            s1_scr = qpool.tile([128, H], F32, name="s1_scr")
            s1_raw = spool.tile([128, 1], F32, name="s1_raw")
            nc.gpsimd.tensor_mul(s1_scr, q_sb, qw_bc)
            nc.vector.tensor_reduce(
                out=s1_raw, in_=s1_scr, axis=mybir.AxisListType.X,
                op=mybir.AluOpType.add,
            )
            s1b = spool.tile([128, 1], F32, name="s1b")
            nc.scalar.activation(out=s1b, in_=s1_raw, func=ident_f, bias=bias_bc, scale=1.0)

            # qT chunks for the pair: pt_q[:, f, 0:64] = b0, [:, f, 64:128] = b1
            pt_q = ps_tr.tile([128, 4, 128], BF16, name="pt_q", tag="trq", bufs=1)
            for f in range(4):
                nc.tensor.transpose(pt_q[:, f, :], qs_bf[:, ts(f, 128)], ident)
            # lhsT[b][f] = [ (qs_b chunk f)^T | cwgt_hi f | junk | cwgt_lo f ]
            # (hi lands in psum row 64, lo in row 96: engine reads need
            # 32-aligned base partitions; rows 65..95 are unused junk)
            for b in (2 * p, 2 * p + 1):
                off = (b % 2) * 64
                lhsT = lpool.tile([128, 4, 97], BF16, name="lhsT")
                nc.vector.tensor_copy(out=lhsT[:, :, 0:64], in_=pt_q[:, :, off:off + 64])
                nc.vector.tensor_copy(out=lhsT[:, :, 64:65], in_=cwgt_hi.rearrange("p (f o) -> p f o", o=1))
                nc.vector.tensor_copy(out=lhsT[:, :, 96:97], in_=cwgt_lo.rearrange("p (f o) -> p f o", o=1))
                S[b]["lhsT"] = lhsT
            # odd batch's q half must sit at base partition 0 for the a-matmul
            # (PE requires lhsT/rhs at the same base partition); DMA it down.
            q_bf1 = qpool.tile([Lq, H], BF16, name="q_bf1")
            nc.scalar.dma_start(out=q_bf1, in_=q_bf[Lq:128, :])
            S[2 * p]["q_bf"] = q_bf[0:Lq, :]
            S[2 * p + 1]["q_bf"] = q_bf1
            P[p].update(s1b=s1b)

        def stage_A(b):
            """per-batch: c cast -> cT transposes -> sT matmuls -> exp"""
            p, off = b // 2, (b % 2) * 64
            c_sb = c_tiles[b]
            c_bf = cbpool.tile([128, 4, H], BF16, name="c_bf")
            nc.scalar.activation(out=c_bf[:, 0, :], in_=c_sb[:, 0, :], func=copy_f)
            nc.vector.tensor_copy(out=c_bf[:, 1, :], in_=c_sb[:, 1, :])
            nc.scalar.activation(out=c_bf[:, 2, :], in_=c_sb[:, 2, :], func=copy_f)
            nc.vector.tensor_copy(out=c_bf[:, 3, :], in_=c_sb[:, 3, :])
            # c passthrough store on the scalar queue (depends only on the load)
            nc.scalar.dma_start(
                out=out_ap[b, :, 0:512].rearrange("(j p) h -> p j h", p=128),
                in_=c_sb,
            )

            # cT[f] = c^T chunk (H rows f*128.., all Lc cols), bf16
            cT = ctpool.tile([128, 4, H], BF16, name="cT")
            for j in range(4):
                pt_c = ps_mm.tile([128, 4, 128], BF16, name="pt_c", tag="big1")
                for f in range(4):
                    nc.tensor.transpose(pt_c[:, f, :], c_bf[:, j, ts(f, 128)], ident)
                if j % 2 == 0:
                    nc.vector.tensor_copy(out=cT[:, :, ts(j, 128)], in_=pt_c)
                else:
                    nc.scalar.activation(out=cT[:, :, ts(j, 128)], in_=pt_c, func=copy_f)

            # sT accumulation: rows 0..63 = qs@cT, rows 64/96 = s0 hi/lo parts
            lhsT = S[b].pop("lhsT")
            ps_sT = ps_mm.tile([128, 512], F32, name="ps_sT", tag="big1", bufs=2)
            for f in range(4):
                nc.tensor.matmul(
                    ps_sT[0:97, :], lhsT[:, f, :], cT[:, f, :],
                    start=(f == 0), stop=False,
                )
            s0hi = spool.tile([1, H], F32, name="s0hi")
            nc.scalar.activation(out=s0hi, in_=ps_sT[64:65, :], func=copy_f)
            s0row = spool.tile([1, H], mybir.dt.float32r, name="s0row")
            nc.vector.tensor_add(s0row, ps_sT[96:97, :], s0hi)
            nc.tensor.matmul(
                ps_sT[0:97, :], aug, s0row,
                start=False, stop=True,
            )

            # E = exp(sT + s1 + bias) in bf16; rowsum (f32) for a2
            E_sb = epool.tile([Lq, H], BF16, name="E_sb")
            rowsum = spool.tile([Lq, 1], F32, name="rowsum")
            nc.scalar.activation(
                out=E_sb, in_=ps_sT[0:64, :], func=exp_f,
                bias=P[p]["s1b"][off:off + 64, :], scale=1.0,
                accum_out=rowsum,
            )
            S[b].update(c_sb=c_sb, c_bf=c_bf, E_sb=E_sb, rowsum=rowsum)

        def stage_B(b):
            """a2 softmax -> a2 transposes -> M2 = a2^T @ c ; colsums"""
            p, off = b // 2, (b % 2) * 64
            c_bf = S[b]["c_bf"]
            E_sb = S[b]["E_sb"]
            ra2 = spool.tile([Lq, 1], F32, name="ra2")
            nc.vector.reciprocal(ra2, S[b]["rowsum"])
            a2T = epool.tile([Lq, H], BF16, name="a2T")
            nc.vector.tensor_scalar_mul(a2T, E_sb, ra2)

            # a2 natural layout [i, j] via PE transposes of a2T
            a2n = btpool.tile([128, 4, Lq], BF16, name="a2n")
            pt_a = ps_tr.tile([128, 4, 64], BF16, name="pt_a", tag="trq", bufs=1)
            for f in range(4):
                nc.tensor.transpose(pt_a[:, f, :], a2T[:, ts(f, 128)], ident[0:64, 0:64])
            nc.vector.tensor_copy(out=a2n, in_=pt_a)

            # M2 = a2^T @ c  [Lq, H]  (b = a1 @ M2 afterwards - associativity)
            ps_M2 = ps_mm.tile([128, 512], F32, name="ps_M2", tag="big1", bufs=2)
            for jj in range(4):
                nc.tensor.matmul(
                    ps_M2[0:64, :], a2n[:, jj, :], c_bf[:, jj, :],
                    start=(jj == 0), stop=(jj == 3),
                )
            M2_bf = epool.tile([Lq, H], BF16, name="M2_bf")
            nc.scalar.activation(out=M2_bf, in_=ps_M2[0:64, :], func=copy_f)

            # column sums of E (normalizer of a1), reciprocal per i-tile
            ps_S = ps_sm.tile([128, 4], F32, name="ps_S")
            for m in range(4):
                nc.tensor.matmul(
                    ps_S[:, m:m + 1], E_sb[:, ts(m, 128)],
                    ones_col[0:Lq, :], start=True, stop=True,
                )
            rS = spool.tile([128, 4], F32, name="rS")
            nc.vector.reciprocal(rS, ps_S)
            S[b].update(rS=rS, M2_bf=M2_bf)

        def stage_C(b, ms):
            """per i-tile: a / b matmuls, scales, products, store"""
            c_sb = S[b]["c_sb"]
            E_sb = S[b]["E_sb"]
            q_bf = S[b]["q_bf"]
            M2_bf = S[b]["M2_bf"]
            rS = S[b]["rS"]
            for m in ms:
                stage = opool.tile([128, 3, H], F32, name="stage")
                ps = ps_ab.tile([128, 2 * H], F32, name="ps", tag="big2")
                nc.tensor.matmul(
                    ps[:, 0:H], E_sb[:, ts(m, 128)], q_bf,
                    start=True, stop=True,
                )
                nc.tensor.matmul(
                    ps[:, H:2 * H], E_sb[:, ts(m, 128)], M2_bf,
                    start=True, stop=True,
                )
                # a = (E^T chunk @ q) * rS ; ca = c * a
                nc.scalar.activation(out=stage[:, 0, :], in_=ps[:, 0:H], func=copy_f, scale=rS[:, m:m + 1])
                if m % 2 == 0:
                    nc.vector.tensor_mul(stage[:, 1, :], stage[:, 0, :], c_sb[:, m, :])
                else:
                    nc.gpsimd.tensor_mul(stage[:, 1, :], stage[:, 0, :], c_sb[:, m, :])
                # b = (a1 @ M2) * rS ; cb = c * b
                nc.vector.scalar_tensor_tensor(
                    out=stage[:, 2, :], in0=ps[:, H:2 * H], scalar=rS[:, m:m + 1],
                    in1=c_sb[:, m, :], op0=MULT, op1=MULT,
                )
                # store: out tile = [a | c*a | c*b]
                nc.sync.dma_start(out=out_ap[b, ts(m, 128), 512:2048], in_=stage)
            if ms[-1] == 3:
                S[b].clear()

        # software-pipelined emission: A(b+2) | B(b+1) | C(b)
        stage_PQ(0)
        stage_A(0)
        stage_B(0)
        stage_A(1)
        stage_C(0, [0, 1])
        stage_B(1)
        stage_C(0, [2, 3])
        stage_PQ(1)
        stage_C(1, [0, 1])
        stage_A(2)
        stage_C(1, [2, 3])
        stage_B(2)
        stage_A(3)
        stage_C(2, [0, 1, 2, 3])
        stage_B(3)
        stage_C(3, [0, 1, 2, 3])

    nc.compile()
    return nc


def _numpy_fallback(c, q, c_mask, q_mask, c_weight, q_weight, cq_weight, bias):
    NEG_INF = -1e30
    s0 = c @ c_weight
    s1 = (q @ q_weight).transpose(0, 2, 1)
    s2 = np.einsum("bih,bjh->bij", c * cq_weight, q)
    s = s0 + s1 + s2 + bias

    def softmax(x, mask, axis):
        logits = np.where(mask, x, NEG_INF)
        m = logits.max(axis=axis, keepdims=True)
        e = np.exp(logits - m)
        return e / e.sum(axis=axis, keepdims=True)

    a1 = softmax(s, q_mask[:, None, :], 2)
    a2 = softmax(s, c_mask[:, :, None], 1)
    a = np.einsum("bij,bjh->bih", a1, q)
    bb = np.einsum("bik,bjk->bij", a1, a2)
    bb = np.einsum("bij,bjh->bih", bb, c)
    return np.concatenate([c, a, c * a, c * bb], axis=2).astype(np.float32)


def kernel(c, q, c_mask, q_mask, c_weight, q_weight, cq_weight, bias, **_):
    c = np.asarray(c, dtype=np.float32)
    q = np.asarray(q, dtype=np.float32)
    if not (np.all(c_mask) and np.all(q_mask)):
        # masks are all-ones per the problem spec; keep a correct fallback
        return _numpy_fallback(
            c, q, np.asarray(c_mask), np.asarray(q_mask),
            np.asarray(c_weight, np.float32), np.asarray(q_weight, np.float32),
            np.asarray(cq_weight, np.float32), np.asarray(bias, np.float32),
        )

    if "nc" not in _CACHE:
        _CACHE["nc"] = _build_program()
    nc = _CACHE["nc"]

    cqw = np.ascontiguousarray(np.asarray(cq_weight, np.float32).reshape(H))
    cwgt = np.ascontiguousarray(np.asarray(c_weight, np.float32).reshape(H))
    qwgt = np.ascontiguousarray(np.asarray(q_weight, np.float32).reshape(H))
    bias_a = np.ascontiguousarray(np.asarray(bias, np.float32).reshape(1))

    in_maps = []
    for k in range(N_CORES):
        in_maps.append(
            {
                "c": np.ascontiguousarray(c[k * BPC : (k + 1) * BPC]),
                "q": np.ascontiguousarray(q[k * BPC : (k + 1) * BPC]),
                "cqw": cqw,
                "cwgt": cwgt,
                "qwgt": qwgt,
                "bias": bias_a,
            }
        )
    res = run_bass_kernel_spmd(nc, in_maps, core_ids=list(range(N_CORES)))
    return np.concatenate([res.results[k]["out"] for k in range(N_CORES)], axis=0)
